# revision 1
# baseline (speedup 1.0000x reference)
"""MoE Transformer layer (attention + top-2 MoE FFN) on TRN2, 8 NeuronCores.

Two SPMD launches:
  A (attention): core c <-> (batch b=c//2, query-half c%2), feature-major layout.
  B (MoE): core e <-> expert e (expert-parallel), capacity-padded token gather.
Host between launches does only sharding work: exact logit affine from device
LN2 stats, top-2 + softmax, per-expert gather (the token dispatch), and the
final scatter-add combine of partial outputs.
"""
import os
import numpy as np

import concourse.bass as bass
import concourse.tile as tile
import concourse.mybir as mybir
from concourse.bass_utils import run_bass_kernel_spmd
from concourse.tile import TileContext, ScopedClock

dt = mybir.dt
AF = mybir.ActivationFunctionType
ALU = mybir.AluOpType

# ---------------------------------------------------------------------------
# Toolchain patch: this walrus rejects >1 semaphore wait per instruction
# ("Too many sync wait commands"). Hoist excess waits onto same-engine NoOp
# carriers; emit kernel-tail drain waits as individual wait instructions.
# ---------------------------------------------------------------------------
_WAIT_CAP = int(os.environ.get("MOE_WAIT_CAP", "1"))
_split_counter = [0]


def _split_waits(ordered):
    for bb_name, insts in ordered.items():
        i = 0
        while i < len(insts):
            inst = insts[i]
            si = inst.sync_info
            if si is not None and len(si.on_wait) > _WAIT_CAP:
                waits = list(si.on_wait)
                keep = waits[-_WAIT_CAP:]
                rest = waits[:-_WAIT_CAP]
                inst.sync_info = mybir.SyncInfo(on_wait=keep, on_update=list(si.on_update))
                carriers = []
                for j in range(0, len(rest), _WAIT_CAP):
                    chunk = rest[j:j + _WAIT_CAP]
                    _split_counter[0] += 1
                    nop = mybir.InstNoOp(name=f"waitsplit-{_split_counter[0]}", ins=[], outs=[])
                    nop.engine = inst.engine
                    nop.sync_info = mybir.SyncInfo(on_wait=chunk, on_update=[])
                    nop.debug = inst.debug
                    carriers.append(nop)
                insts[i:i] = carriers
                i += len(carriers)
            i += 1


_orig_lower_ordered = TileContext._lower_ordered_insts


def _patched_lower_ordered(self, ordered):
    _split_waits(ordered)
    return _orig_lower_ordered(self, ordered)


def _patched_drain_and_barrier(self, tick_clock, wait_clock):
    probe = self.nc.sync.nop(nofuse=True, hint="drain_waits_probe")
    wait_clock.add_sem_waits(probe.ins, ScopedClock({None: tick_clock.global_clock}))
    si = probe.ins.sync_info
    waits = list(si.on_wait) if si is not None else []
    if si is not None:
        probe.ins.sync_info = mybir.SyncInfo(on_wait=[], on_update=list(si.on_update))
    assert self.sems is not None
    allocated = self.sems.allocated()
    by_name = {}
    for k, h in allocated.items():
        name = getattr(h, "name", None) or str(k)
        by_name[name] = h
    for w in waits:
        h = by_name.get(w.ant_name)
        if h is None:
            for hh in allocated.values():
                if getattr(hh, "index", None) == w.id or getattr(hh, "id", None) == w.id:
                    h = hh
                    break
        assert h is not None, f"no semaphore handle for {w.ant_name}"
        assert w.wait_mode == "sem-ge-imm", w.wait_mode
        self.nc.sync.wait_ge(h, w.wait_value)
    self.nc.sync.drain()

    self.nc.all_engine_barrier()
    popped = self.nc._tile_sem_poison_stack.pop()
    assert popped is self._sem_poison
    self.nc.clear_and_free_semaphores(list(self.sems.allocated().values()))
    self.nc.all_engine_barrier()


if not getattr(TileContext, "_moe_patched", False):
    TileContext._lower_ordered_insts = _patched_lower_ordered
    TileContext._drain_and_barrier = _patched_drain_and_barrier
    TileContext._moe_patched = True

# ---------------------------------------------------------------------------
# Problem constants (hardcoded per contract)
# ---------------------------------------------------------------------------
S, B, E, H, HD, FF, NE = 2048, 4, 1024, 16, 64, 4096, 8
LN_EPS = 1e-5
P = 128
EC = E // P           # 8 E-chunks of 128
FT = FF // P          # 32 FF-chunks of 128
TOK = 2048            # tokens per core in launch A (one batch)
Q = 1024              # query (owned) tokens per core
KC = TOK // P         # 16 key chunks
CT = 17               # capacity tiles for launch B
C = CT * P            # 2176 token capacity per expert
NCORES = 8

_cache = {}


def _mm(nc, psum_ap, lhsT, rhs, start, stop):
    """matmul with the moving dim split into <=512 column slices."""
    n = rhs.shape[-1]
    for off in range(0, n, 512):
        sl = slice(off, min(off + 512, n))
        nc.tensor.matmul(psum_ap[..., sl], lhsT, rhs[..., sl], start=start, stop=stop)


# ---------------------------------------------------------------------------
# Launch A: LN1 -> QKV -> attention -> out-proj(+residual) -> LN2 stats + gate
# ---------------------------------------------------------------------------
def _build_A(cut="all", ln1_triv=False, ln2_triv=False, outb_zero=False):
    nc = bass.Bass("TRN2", target_bir_lowering=False, debug=False)

    xqT = nc.dram_tensor("xqT", [P, EC, Q], dt.float32, kind="ExternalInput").ap()
    xoT = nc.dram_tensor("xoT", [P, EC, Q], dt.float32, kind="ExternalInput").ap()
    wqkvT = nc.dram_tensor("wqkvT", [P, EC, 3 * E], dt.float16, kind="ExternalInput").ap()
    owp = nc.dram_tensor("owp", [P, H, E], dt.float16, kind="ExternalInput").ap()
    gT = nc.dram_tensor("gT", [P, EC, NE], dt.float32, kind="ExternalInput").ap()
    ln1g = nc.dram_tensor("ln1g", [P, EC], dt.float32, kind="ExternalInput").ap()
    ln1b = nc.dram_tensor("ln1b", [P, EC], dt.float32, kind="ExternalInput").ap()
    ln2g = nc.dram_tensor("ln2g", [P, EC], dt.float32, kind="ExternalInput").ap()
    ln2b = nc.dram_tensor("ln2b", [P, EC], dt.float32, kind="ExternalInput").ap()
    outb = nc.dram_tensor("outb", [P, EC], dt.float32, kind="ExternalInput").ap()

    x1T_o = nc.dram_tensor("x1T", [P, EC, Q], dt.float32, kind="ExternalOutput").ap()
    xn2T_o = nc.dram_tensor("xn2T", [P, EC, Q], dt.float16, kind="ExternalOutput").ap()
    lgT_o = nc.dram_tensor("lgT", [NE, Q], dt.float32, kind="ExternalOutput").ap()
    mu2_o = nc.dram_tensor("mu2", [1, Q], dt.float32, kind="ExternalOutput").ap()
    rstd2_o = nc.dram_tensor("rstd2", [1, Q], dt.float32, kind="ExternalOutput").ap()

    with TileContext(nc) as tc:
        const = tc.alloc_tile_pool(name="const", bufs=1)
        ones128 = const.tile([P, 1], dt.float32)
        nc.vector.memset(ones128[:], 1.0)
        eps1 = const.tile([1, 1], dt.float32)
        nc.vector.memset(eps1[:], LN_EPS)
        ones_row = const.tile([1, P], dt.float32)
        nc.vector.memset(ones_row[:], 1.0)
        g1 = const.tile([P, EC], dt.float32)
        nc.sync.dma_start(g1[:], ln1g)
        b1 = const.tile([P, EC], dt.float32)
        nc.sync.dma_start(b1[:], ln1b)
        g2 = const.tile([P, EC], dt.float32)
        nc.sync.dma_start(g2[:], ln2g)
        b2 = const.tile([P, EC], dt.float32)
        nc.sync.dma_start(b2[:], ln2b)
        ob = const.tile([P, EC], dt.float32)
        nc.sync.dma_start(ob[:], outb)

        # QKV outputs — released after attention
        p_av = tc.alloc_tile_pool(name="p_av", bufs=1)
        qT = p_av.tile([P, EC, Q], dt.float16)
        kT = p_av.tile([P, EC, TOK], dt.float16)
        vaug = p_av.tile([P, KC, H * (HD + 1)], dt.float16)
        va = vaug[:].rearrange("p t (h w) -> p t h w", w=HD + 1)
        nc.vector.memset(va[:, :, :, HD:HD + 1], 1.0)

        # ---- phase 1: LN1 (stats via fp32 ones-matmuls, apply on DVE) ----
        p_ln = tc.alloc_tile_pool(name="p_ln", bufs=1)
        xnT = p_ln.tile([P, EC, TOK], dt.float16)
        p_lt = tc.alloc_tile_pool(name="p_lt", bufs=1)
        stats = p_lt.tile([1, 3, TOK], dt.float32)
        mu_s = p_lt.tile([P, TOK], dt.float32)
        rs_s = p_lt.tile([P, TOK], dt.float32)
        p_xs = tc.alloc_tile_pool(name="p_xs", bufs=3)
        p_sq = tc.alloc_tile_pool(name="p_sq", bufs=2)

        ps_st = tc.alloc_tile_pool(name="ps_st", bufs=1, space="PSUM")
        musum = ps_st.tile([1, TOK], dt.float32, tag="musum")
        sqsum = ps_st.tile([1, TOK], dt.float32, tag="sqsum")
        for c in range(EC):
            for src, cols in ((xqT, slice(0, Q)), (xoT, slice(Q, TOK))):
                xc = p_xs.tile([P, Q], dt.float32, tag="xs")
                nc.sync.dma_start(xc[:], src[:, c, :])
                _mm(nc, musum[:, cols], ones128[:], xc[:], c == 0, c == EC - 1)
                sq = p_sq.tile([P, Q], dt.float32, tag="sq")
                nc.vector.tensor_mul(sq[:], xc[:], xc[:])
                _mm(nc, sqsum[:, cols], ones128[:], sq[:], c == 0, c == EC - 1)
        nc.vector.tensor_scalar_mul(stats[:, 0, :], musum[:], 1.0 / E)
        nc.vector.tensor_scalar_mul(stats[:, 1, :], sqsum[:], 1.0 / E)
        nc.vector.tensor_mul(stats[:, 2, :], stats[:, 0, :], stats[:, 0, :])
        nc.vector.tensor_sub(stats[:, 1, :], stats[:, 1, :], stats[:, 2, :])
        nc.scalar.activation(stats[:, 1, :], stats[:, 1, :], AF.Sqrt, bias=eps1[:])
        nc.vector.reciprocal(stats[:, 1, :], stats[:, 1, :])
        ps_st.release()

        ps_bc = tc.alloc_tile_pool(name="ps_bc", bufs=1, space="PSUM")
        mub = ps_bc.tile([P, TOK], dt.float32, tag="mub")
        rsb = ps_bc.tile([P, TOK], dt.float32, tag="rsb")
        _mm(nc, mub[:], ones_row[:], stats[:, 0, :], True, True)
        _mm(nc, rsb[:], ones_row[:], stats[:, 1, :], True, True)
        nc.vector.tensor_copy(mu_s[:], mub[:])
        nc.vector.tensor_copy(rs_s[:], rsb[:])
        ps_bc.release()

        p_ap = tc.alloc_tile_pool(name="p_ap", bufs=3)
        for c in range(EC):
            for src, cols in ((xqT, slice(0, Q)), (xoT, slice(Q, TOK))):
                xc = p_xs.tile([P, Q], dt.float32, tag="xs")
                nc.sync.dma_start(xc[:], src[:, c, :])
                t = p_ap.tile([P, Q], dt.float32, tag="ap")
                nc.vector.tensor_sub(t[:], xc[:], mu_s[:, cols])
                if ln1_triv:
                    nc.vector.tensor_mul(xnT[:, c, cols], t[:], rs_s[:, cols])
                else:
                    nc.vector.tensor_mul(t[:], t[:], rs_s[:, cols])
                    nc.vector.tensor_scalar(
                        xnT[:, c, cols], t[:], g1[:, c:c + 1], b1[:, c:c + 1],
                        op0=ALU.mult, op1=ALU.add)
        p_ap.release()
        p_sq.release()
        p_xs.release()
        p_lt.release()
        if cut == "ln1":
            p_ln.release(); p_av.release(); const.release()
            return nc

        # ---- phase 2: QKV (v first so attention can start during k/q) ----
        ps_qkv = tc.alloc_tile_pool(name="ps_qkv", bufs=4, space="PSUM")
        p_wv = tc.alloc_tile_pool(name="p_wv", bufs=1)
        wv = p_wv.tile([P, EC, E], dt.float16)
        for c in range(EC):
            nc.sync.dma_start(wv[:, c, :], wqkvT[:, c, 2 * E:3 * E])
        for tt in range(KC):           # v in token-major layout -> vaug
            for half in range(2):
                sl = slice(half * 512, half * 512 + 512)
                pv = ps_qkv.tile([P, 512], dt.float32, tag="pq")
                for c in range(EC):
                    nc.tensor.matmul(pv[:], xnT[:, c, tt * P:(tt + 1) * P],
                                     wv[:, c, sl],
                                     start=(c == 0), stop=(c == EC - 1))
                nc.any.tensor_copy(
                    va[:, tt, half * 8:(half + 1) * 8, 0:HD],
                    pv[:].rearrange("p (h d) -> p h d", d=HD))

        p_wqk = tc.alloc_tile_pool(name="p_wqk", bufs=1)
        wqk = p_wqk.tile([P, EC, 2 * E], dt.float16)
        for c in range(EC):
            nc.sync.dma_start(wqk[:, c, :], wqkvT[:, c, 0:2 * E])
        for ft in range(EC):           # per head-pair: k (all tokens) then q
            for quad in range(4):
                sl = slice(quad * 512, quad * 512 + 512)
                pk = ps_qkv.tile([P, 512], dt.float32, tag="pq")
                for c in range(EC):
                    nc.tensor.matmul(pk[:], wqk[:, c, E + ft * P:E + (ft + 1) * P],
                                     xnT[:, c, sl],
                                     start=(c == 0), stop=(c == EC - 1))
                nc.any.tensor_copy(kT[:, ft, sl], pk[:])
            for half in range(2):
                sl = slice(half * 512, half * 512 + 512)
                pq = ps_qkv.tile([P, 512], dt.float32, tag="pq")
                for c in range(EC):
                    nc.tensor.matmul(pq[:], wqk[:, c, ft * P:(ft + 1) * P],
                                     xnT[:, c, 0:Q][:, sl],
                                     start=(c == 0), stop=(c == EC - 1))
                nc.any.tensor_copy(qT[:, ft, sl], pq[:])
        p_wqk.release()
        p_wv.release()
        p_ln.release()
        if cut == "qkv":
            ps_qkv.release(); p_av.release(); const.release()
            return nc

        # ---- phase 3: attention ----
        ps_qkv.release()
        p_ctx = tc.alloc_tile_pool(name="p_ctx", bufs=1, side="right")
        ctxT = p_ctx.tile([P, H, Q], dt.float16)
        nc.vector.memset(ctxT[64:128, :, :], 0.0)
        ps_sc = tc.alloc_tile_pool(name="ps_sc", bufs=1, space="PSUM")
        ps_ct = tc.alloc_tile_pool(name="ps_ct", bufs=1, space="PSUM")
        p_pr = tc.alloc_tile_pool(name="p_pr", bufs=8)
        p_dv = tc.alloc_tile_pool(name="p_dv", bufs=2)
        for hp in range(H // 2):
            sc = [ps_sc.tile([P, Q], dt.float32, tag=f"sc{j}", name=f"sc{j}") for j in range(2)]
            ct = [ps_ct.tile([65, Q], dt.float32, tag=f"ct{j}", name=f"ct{j}") for j in range(2)]
            for kc in range(KC):
                pr = []
                for j in range(2):
                    lo, hi = 64 * j, 64 * (j + 1)
                    _mm(nc, sc[j][:], kT[lo:hi, hp, kc * P:(kc + 1) * P],
                        qT[lo:hi, hp, :], True, True)
                    prj = p_pr.tile([P, Q], dt.float16, tag="pr", name="prj")
                    nc.scalar.activation(prj[:], sc[j][:], AF.Exp)
                    pr.append(prj)
                for j in range(2):
                    _mm(nc, ct[j][:], va[:, kc, 2 * hp + j, :], pr[j][:],
                        kc == 0, kc == KC - 1)
            for j in range(2):
                h = 2 * hp + j
                rec = p_dv.tile([1, Q], dt.float32, tag="rec")
                nc.vector.reciprocal(rec[:], ct[j][64:65, :])
                rb = ps_sc.tile([64, Q], dt.float32, tag=f"sc{j}", name=f"rb{j}")
                _mm(nc, rb[:], ones_row[:, 0:64], rec[:], True, True)
                rbs = p_dv.tile([64, Q], dt.float32, tag="rbs")
                nc.vector.tensor_copy(rbs[:], rb[:])
                nc.vector.tensor_mul(ctxT[0:64, h, :], ct[j][0:64, :], rbs[:])
        p_dv.release()
        p_pr.release()
        ps_ct.release()
        ps_sc.release()
        p_av.release()
        if cut == "attn":
            p_ctx.release(); const.release()
            return nc

        # ---- phase 4: out-proj + residual ----
        p_x1 = tc.alloc_tile_pool(name="p_x1", bufs=1)
        x1T = p_x1.tile([P, EC, Q], dt.float32)
        p_ow = tc.alloc_tile_pool(name="p_ow", bufs=1)
        ow = p_ow.tile([P, H, E], dt.float16)
        for h in range(H):
            nc.sync.dma_start(ow[:, h, :], owp[:, h, :])
        p_xr = tc.alloc_tile_pool(name="p_xr", bufs=3)
        ps_ao = tc.alloc_tile_pool(name="ps_ao", bufs=2, space="PSUM")
        for eo in range(EC):
            ao = ps_ao.tile([P, Q], dt.float32, tag="ao")
            for h in range(H):
                _mm(nc, ao[:], ow[:, h, eo * P:(eo + 1) * P], ctxT[:, h, :],
                    h == 0, h == H - 1)
            xc = p_xr.tile([P, Q], dt.float32, tag="xr")
            nc.sync.dma_start(xc[:], xqT[:, eo, :])
            nc.vector.tensor_add(x1T[:, eo, :], ao[:], xc[:])
            if not outb_zero:
                nc.vector.tensor_scalar(
                    x1T[:, eo, :], x1T[:, eo, :], ob[:, eo:eo + 1], None, op0=ALU.add)
            nc.sync.dma_start(x1T_o[:, eo, :], x1T[:, eo, :])
        ps_ao.release()
        p_xr.release()
        p_ow.release()
        p_ctx.release()
        if cut == "oproj":
            p_x1.release(); const.release()
            return nc

        # ---- phase 5: LN2 stats + gate logits + xn2T ----
        p_l2 = tc.alloc_tile_pool(name="p_l2", bufs=1)
        st2 = p_l2.tile([1, 3, Q], dt.float32)
        gts = p_l2.tile([P, EC, NE], dt.float32)
        nc.sync.dma_start(gts[:], gT)
        lgs = p_l2.tile([NE, Q], dt.float32)
        mu2s = p_l2.tile([P, Q], dt.float32)
        rs2s = p_l2.tile([P, Q], dt.float32)

        ps_s2 = tc.alloc_tile_pool(name="ps_s2", bufs=1, space="PSUM")
        musum2 = ps_s2.tile([1, Q], dt.float32, tag="musum2")
        sqsum2 = ps_s2.tile([1, Q], dt.float32, tag="sqsum2")
        lgp = ps_s2.tile([NE, Q], dt.float32, tag="lgp")
        p_q2 = tc.alloc_tile_pool(name="p_q2", bufs=2)
        for c in range(EC):
            _mm(nc, musum2[:], ones128[:], x1T[:, c, :], c == 0, c == EC - 1)
            sq = p_q2.tile([P, Q], dt.float32, tag="sq2")
            nc.vector.tensor_mul(sq[:], x1T[:, c, :], x1T[:, c, :])
            _mm(nc, sqsum2[:], ones128[:], sq[:], c == 0, c == EC - 1)
            _mm(nc, lgp[:], gts[:, c, :], x1T[:, c, :], c == 0, c == EC - 1)
        nc.vector.tensor_scalar_mul(st2[:, 0, :], musum2[:], 1.0 / E)
        nc.vector.tensor_scalar_mul(st2[:, 1, :], sqsum2[:], 1.0 / E)
        nc.vector.tensor_mul(st2[:, 2, :], st2[:, 0, :], st2[:, 0, :])
        nc.vector.tensor_sub(st2[:, 1, :], st2[:, 1, :], st2[:, 2, :])
        nc.scalar.activation(st2[:, 1, :], st2[:, 1, :], AF.Sqrt, bias=eps1[:])
        nc.vector.reciprocal(st2[:, 1, :], st2[:, 1, :])
        nc.vector.tensor_copy(lgs[:], lgp[:])
        nc.sync.dma_start(lgT_o, lgs[:])
        nc.sync.dma_start(mu2_o, st2[:, 0, :])
        nc.sync.dma_start(rstd2_o, st2[:, 1, :])
        p_q2.release()
        ps_s2.release()

        ps_b2 = tc.alloc_tile_pool(name="ps_b2", bufs=1, space="PSUM")
        mub2 = ps_b2.tile([P, Q], dt.float32, tag="mub2")
        rsb2 = ps_b2.tile([P, Q], dt.float32, tag="rsb2")
        _mm(nc, mub2[:], ones_row[:], st2[:, 0, :], True, True)
        _mm(nc, rsb2[:], ones_row[:], st2[:, 1, :], True, True)
        nc.vector.tensor_copy(mu2s[:], mub2[:])
        nc.vector.tensor_copy(rs2s[:], rsb2[:])
        ps_b2.release()

        p_x2 = tc.alloc_tile_pool(name="p_x2", bufs=3)
        for c in range(EC):
            t = p_x2.tile([P, Q], dt.float32, tag="x2t")
            nc.vector.tensor_sub(t[:], x1T[:, c, :], mu2s[:])
            t16 = p_x2.tile([P, Q], dt.float16, tag="x2t16")
            if ln2_triv:
                nc.vector.tensor_mul(t16[:], t[:], rs2s[:])
            else:
                nc.vector.tensor_mul(t[:], t[:], rs2s[:])
                nc.vector.tensor_scalar(
                    t16[:], t[:], g2[:, c:c + 1], b2[:, c:c + 1],
                    op0=ALU.mult, op1=ALU.add)
            nc.sync.dma_start(xn2T_o[:, c, :], t16[:])
        p_x2.release()
        p_l2.release()
        p_x1.release()
        const.release()

    return nc


# ---------------------------------------------------------------------------
# Launch B: expert FFN, hT = gelu(w1^T x + b1) in ff-major, o = hT^T w2
# ---------------------------------------------------------------------------
def _build_B(ntt_max=2, op_bufs=1):
    nc = bass.Bass("TRN2", target_bir_lowering=False, debug=False)
    xeT = nc.dram_tensor("xeT", [P, EC, C], dt.float16, kind="ExternalInput").ap()
    w1e = nc.dram_tensor("w1e", [P, EC, FF], dt.float16, kind="ExternalInput").ap()
    w2e = nc.dram_tensor("w2e", [P, FT, E], dt.float16, kind="ExternalInput").ap()
    b1e = nc.dram_tensor("b1e", [P, FT], dt.float32, kind="ExternalInput").ap()
    wcm = nc.dram_tensor("wcm", [P, CT], dt.float32, kind="ExternalInput").ap()
    o_out = nc.dram_tensor("o", [P, CT, E], dt.float32, kind="ExternalOutput").ap()

    with TileContext(nc) as tc:
        sb = tc.alloc_tile_pool(name="sb", bufs=1)
        xe = sb.tile([P, EC, C], dt.float16)
        for c in range(EC):
            nc.sync.dma_start(xe[:, c, :], xeT[:, c, :])
        w1 = sb.tile([P, EC, FF], dt.float16)
        for c in range(EC):
            for fh in range(4):
                nc.sync.dma_start(w1[:, c, fh * FF // 4:(fh + 1) * FF // 4],
                                  w1e[:, c, fh * FF // 4:(fh + 1) * FF // 4])
        w2 = sb.tile([P, FT, E], dt.float16)
        for fc in range(FT):
            nc.sync.dma_start(w2[:, fc, :], w2e[:, fc, :])
        bb = sb.tile([P, FT], dt.float32)
        nc.sync.dma_start(bb[:], b1e)
        wc = sb.tile([P, CT], dt.float32)
        nc.sync.dma_start(wc[:], wcm)

        hp_pool = tc.alloc_tile_pool(name="hp", bufs=2, space="PSUM")
        op_pool = tc.alloc_tile_pool(name="op", bufs=op_bufs, space="PSUM")
        hs_pool = tc.alloc_tile_pool(name="hs", bufs=3)
        os_pool = tc.alloc_tile_pool(name="os", bufs=3)

        t0 = 0
        while t0 < CT:
            ntt = min(ntt_max, CT - t0)
            ops = [op_pool.tile([P, E], dt.float32, tag=f"o{i}", name=f"o{i}") for i in range(ntt)]
            for fc in range(FT):
                hps = hp_pool.tile([P, ntt * P], dt.float32, tag="h")
                for c in range(EC):
                    _mm(nc, hps[:], w1[:, c, fc * P:(fc + 1) * P],
                        xe[:, c, t0 * P:(t0 + ntt) * P], c == 0, c == EC - 1)
                hs = hs_pool.tile([P, ntt * P], dt.float16, tag="hs")
                nc.scalar.activation(hs[:], hps[:], AF.Gelu, bias=bb[:, fc:fc + 1])
                for i in range(ntt):
                    _mm(nc, ops[i][:], hs[:, i * P:(i + 1) * P], w2[:, fc, :],
                        fc == 0, fc == FT - 1)
            for i in range(ntt):
                osb = os_pool.tile([P, E], dt.float32, tag="osb")
                nc.vector.tensor_scalar_mul(osb[:], ops[i][:], wc[:, t0 + i:t0 + i + 1])
                nc.sync.dma_start(o_out[:, t0 + i, :], osb[:])
            t0 += ntt

        os_pool.release()
        hs_pool.release()
        op_pool.release()
        hp_pool.release()
        sb.release()

    return nc


# ---------------------------------------------------------------------------
# Host-side helpers
# ---------------------------------------------------------------------------
def _chunkE(a):
    """[E, T] -> [P, EC, T]"""
    return np.ascontiguousarray(a.reshape(EC, P, -1).transpose(1, 0, 2))


def _vecE(a):
    """[E] -> [P, EC] with element (p, c) = a[c*P + p]"""
    return np.ascontiguousarray(a.reshape(-1, P).T)


def kernel(**inputs):
    x = np.asarray(inputs["x"], dtype=np.float32)
    in_proj_w = np.asarray(inputs["in_proj_w"], dtype=np.float32)
    in_proj_b = np.asarray(inputs["in_proj_b"], dtype=np.float32)
    out_w = np.asarray(inputs["out_w"], dtype=np.float32)
    out_b = np.asarray(inputs["out_b"], dtype=np.float32)
    ln1_g = np.asarray(inputs["ln1_g"], dtype=np.float32)
    ln1_b = np.asarray(inputs["ln1_b"], dtype=np.float32)
    ln2_g = np.asarray(inputs["ln2_g"], dtype=np.float32)
    ln2_b = np.asarray(inputs["ln2_b"], dtype=np.float32)
    gate_w = np.asarray(inputs["gate_w"], dtype=np.float32)
    gate_b = np.asarray(inputs["gate_b"], dtype=np.float32)
    w1 = np.asarray(inputs["w1"], dtype=np.float32)
    b1 = np.asarray(inputs["b1"], dtype=np.float32)
    w2 = np.asarray(inputs["w2"], dtype=np.float32)
    b2 = np.asarray(inputs["b2"], dtype=np.float32)

    assert np.all(in_proj_b == 0.0), "nonzero in_proj_b unsupported"

    trace = bool(os.environ.get("MOE_TRACE"))

    ln1_triv = bool(np.all(ln1_g == 1.0) and np.all(ln1_b == 0.0))
    ln2_triv = bool(np.all(ln2_g == 1.0) and np.all(ln2_b == 0.0))
    outb_zero = bool(np.all(out_b == 0.0))
    akey = ("A", ln1_triv, ln2_triv, outb_zero)
    if akey not in _cache:
        _cache[akey] = _build_A(ln1_triv=ln1_triv, ln2_triv=ln2_triv, outb_zero=outb_zero)
    if "B" not in _cache:
        _cache["B"] = _build_B()
    ncA, ncB = _cache[akey], _cache["B"]

    # ---- launch A host prep (pure reshard / fold) ----
    wqkvT = in_proj_w.T.copy()              # [E, 3E]
    wqkvT[:, 0:E] *= 1.0 / np.sqrt(HD)      # fold q scaling
    wqkvT16 = _chunkE(wqkvT).astype(np.float16)

    owp = np.zeros((P, H, E), dtype=np.float16)
    for h in range(H):
        owp[0:64, h, :] = out_w[:, h * 64:(h + 1) * 64].T.astype(np.float16)

    G = (gate_w.astype(np.float64) * ln2_g.astype(np.float64)[None, :])   # [NE, E]
    gT = _chunkE(np.ascontiguousarray(G.T).astype(np.float32))
    SG = G.sum(axis=1)
    CB = (ln2_b.astype(np.float64)[None, :] * gate_w.astype(np.float64)).sum(axis=1) \
        + gate_b.astype(np.float64)

    shared = {
        "wqkvT": wqkvT16, "owp": owp, "gT": gT,
        "ln1g": _vecE(ln1_g), "ln1b": _vecE(ln1_b),
        "ln2g": _vecE(ln2_g), "ln2b": _vecE(ln2_b), "outb": _vecE(out_b),
    }

    in_maps_A = []
    for c in range(NCORES):
        b, qh = c // 2, c % 2
        xT = x[:, b, :].T                                    # [E, S]
        xqT = _chunkE(np.ascontiguousarray(xT[:, qh * Q:(qh + 1) * Q]))
        xoT = _chunkE(np.ascontiguousarray(xT[:, (1 - qh) * Q:(2 - qh) * Q]))
        in_maps_A.append({"xqT": xqT, "xoT": xoT, **shared})

    resA = run_bass_kernel_spmd(ncA, in_maps_A, core_ids=list(range(NCORES)), trace=trace)
    outsA = resA.results
    if trace:
        _cache["resA"] = resA

    # ---- host routing (exact logits from device raw + LN2 stats) ----
    T = S * B
    x1_all = np.empty((T, E), dtype=np.float32)
    xn2T_all = np.empty((E, T), dtype=np.float16)
    logits = np.empty((T, NE), dtype=np.float64)
    for c in range(NCORES):
        b, qh = c // 2, c % 2
        r = outsA[c]
        rows = np.arange(qh * Q, (qh + 1) * Q) * B + b        # global token ids
        x1T = r["x1T"].transpose(1, 0, 2).reshape(E, Q)
        x1_all[rows] = x1T.T
        xn2T_all[:, rows] = r["xn2T"].transpose(1, 0, 2).reshape(E, Q)
        raw = r["lgT"].astype(np.float64)                     # [NE, Q]
        mu = r["mu2"][0].astype(np.float64)
        rstd = r["rstd2"][0].astype(np.float64)
        logits[rows] = (raw * rstd[None, :] - (rstd * mu)[None, :] * SG[:, None]
                        + CB[:, None]).T

    idx1 = np.argmax(logits, axis=1)
    l2m = logits.copy()
    l2m[np.arange(T), idx1] = -np.inf
    idx2 = np.argmax(l2m, axis=1)
    v1 = logits[np.arange(T), idx1]
    v2 = logits[np.arange(T), idx2]
    e2 = np.exp(v2 - v1)
    gsc1 = (1.0 / (1.0 + e2)).astype(np.float32)
    gsc2 = (e2 / (1.0 + e2)).astype(np.float32)

    expert_rows, expert_w = [], []
    for e in range(NE):
        m1 = idx1 == e
        m2 = idx2 == e
        rows = np.nonzero(m1 | m2)[0]
        w = np.where(m1[rows], gsc1[rows], gsc2[rows]).astype(np.float32)
        if len(rows) > C:   # capacity safeguard: drop lowest-weight assignments
            keep = np.sort(np.argsort(-w)[:C])
            rows, w = rows[keep], w[keep]
        expert_rows.append(rows)
        expert_w.append(w)

    in_maps_B = []
    for e in range(NE):
        rows, w = expert_rows[e], expert_w[e]
        xeT = np.zeros((E, C), dtype=np.float16)
        xeT[:, :len(rows)] = xn2T_all[:, rows]
        wcmv = np.zeros(C, dtype=np.float32)
        wcmv[:len(rows)] = w
        in_maps_B.append({
            "xeT": _chunkE(xeT),
            "w1e": _chunkE(w1[e]).astype(np.float16),
            "w2e": np.ascontiguousarray(
                w2[e].reshape(FT, P, E).transpose(1, 0, 2)).astype(np.float16),
            "b1e": np.ascontiguousarray(b1[e].reshape(FT, P).T),
            "wcm": np.ascontiguousarray(wcmv.reshape(CT, P).T),
        })

    resB = run_bass_kernel_spmd(ncB, in_maps_B, core_ids=list(range(NCORES)), trace=trace)
    outsB = resB.results
    if trace:
        _cache["resB"] = resB

    # ---- combine (unshard of partial outputs) ----
    y = np.zeros((T, E), dtype=np.float32)
    for e in range(NE):
        rows, w = expert_rows[e], expert_w[e]
        o = outsB[e]["o"].transpose(1, 0, 2).reshape(C, E)
        y[rows] += o[:len(rows)]
        if np.any(b2[e] != 0.0):
            y[rows] += w[:, None] * b2[e][None, :]

    return (x1_all + y).reshape(S, B, E)



# revision 3
# speedup vs baseline: 2.4566x; 2.4566x over previous
"""MoE Transformer layer (attention + top-2 MoE FFN) on TRN2, 8 NeuronCores.

v2: fp8 DoubleRow matmuls everywhere, host-side LN/routing glue.

Launch A (attention): core c = 2*batch + head_group; each core computes its
8 heads over all 2048 tokens of its batch: QKV -> scores (DR-32 per head)
-> exp (ACT exact / DVE Schraudolph-to-fp8) -> AV (DR-128, ones-augmented
denominator) -> normalize -> partial out-proj (DR). Host: LN1 before, residual
+ LN2 + exact float64 top-2 routing after.

Launch B (MoE FFN): core e = expert e; capacity-padded token batch, fp8 DR
GEMMs with optional hi/lo splits for accuracy; host combines.
"""
import os
import numpy as np
import ml_dtypes

import concourse.bass as bass
import concourse.mybir as mybir
from concourse.bass_utils import run_bass_kernel_spmd
from concourse.tile import TileContext, ScopedClock

dt = mybir.dt
AF = mybir.ActivationFunctionType
ALU = mybir.AluOpType
PM = mybir.MatmulPerfMode
F8 = ml_dtypes.float8_e4m3

# ---------------------------------------------------------------------------
# Toolchain patch (from baseline): walrus rejects >1 semaphore wait per
# instruction; hoist excess waits onto same-engine NoOp carriers.
# ---------------------------------------------------------------------------
_WAIT_CAP = int(os.environ.get("MOE_WAIT_CAP", "1"))
_split_counter = [0]


def _split_waits(ordered):
    for bb_name, insts in ordered.items():
        i = 0
        while i < len(insts):
            inst = insts[i]
            si = inst.sync_info
            if si is not None and len(si.on_wait) > _WAIT_CAP:
                waits = list(si.on_wait)
                keep = waits[-_WAIT_CAP:]
                rest = waits[:-_WAIT_CAP]
                inst.sync_info = mybir.SyncInfo(on_wait=keep, on_update=list(si.on_update))
                carriers = []
                for j in range(0, len(rest), _WAIT_CAP):
                    chunk = rest[j:j + _WAIT_CAP]
                    _split_counter[0] += 1
                    nop = mybir.InstNoOp(name=f"waitsplit-{_split_counter[0]}", ins=[], outs=[])
                    nop.engine = inst.engine
                    nop.sync_info = mybir.SyncInfo(on_wait=chunk, on_update=[])
                    nop.debug = inst.debug
                    carriers.append(nop)
                insts[i:i] = carriers
                i += len(carriers)
            i += 1


_orig_lower_ordered = TileContext._lower_ordered_insts


def _patched_lower_ordered(self, ordered):
    _split_waits(ordered)
    return _orig_lower_ordered(self, ordered)


def _patched_drain_and_barrier(self, tick_clock, wait_clock):
    probe = self.nc.sync.nop(nofuse=True, hint="drain_waits_probe")
    wait_clock.add_sem_waits(probe.ins, ScopedClock({None: tick_clock.global_clock}))
    si = probe.ins.sync_info
    waits = list(si.on_wait) if si is not None else []
    if si is not None:
        probe.ins.sync_info = mybir.SyncInfo(on_wait=[], on_update=list(si.on_update))
    assert self.sems is not None
    allocated = self.sems.allocated()
    by_name = {}
    for k, h in allocated.items():
        name = getattr(h, "name", None) or str(k)
        by_name[name] = h
    for w in waits:
        h = by_name.get(w.ant_name)
        if h is None:
            for hh in allocated.values():
                if getattr(hh, "index", None) == w.id or getattr(hh, "id", None) == w.id:
                    h = hh
                    break
        assert h is not None, f"no semaphore handle for {w.ant_name}"
        assert w.wait_mode == "sem-ge-imm", w.wait_mode
        self.nc.sync.wait_ge(h, w.wait_value)
    self.nc.sync.drain()

    self.nc.all_engine_barrier()
    popped = self.nc._tile_sem_poison_stack.pop()
    assert popped is self._sem_poison
    self.nc.clear_and_free_semaphores(list(self.sems.allocated().values()))
    self.nc.all_engine_barrier()


if not getattr(TileContext, "_moe_patched", False):
    TileContext._lower_ordered_insts = _patched_lower_ordered
    TileContext._drain_and_barrier = _patched_drain_and_barrier
    TileContext._moe_patched = True

# ---------------------------------------------------------------------------
# Problem constants
# ---------------------------------------------------------------------------
S, B, E, H, HD, FF, NE = 2048, 4, 1024, 16, 64, 4096, 8
LN_EPS = 1e-5
P = 128
TOK = 2048            # tokens per batch (= keys = queries per core)
HL = 8                # heads per core (head-group split)
KC = TOK // P         # 16 key chunks
KCP = KC // 2         # 8 key chunk pairs (DoubleRow)
CT = 17               # capacity tiles for launch B
C = CT * P            # 2176 token capacity per expert
NCORES = 8
LOG2E = float(np.log2(np.e))

# precision toggles
QK16 = bool(int(os.environ.get("MOE_QK16", "1")))       # q/k in fp16 (scores non-DR)
XESPLIT = bool(int(os.environ.get("MOE_XESPLIT", "1"))) # hi/lo split of xn2 into w1 GEMM
W2SPLIT = bool(int(os.environ.get("MOE_W2SPLIT", "1"))) # hi/lo split of w2 (extra psum + host add)
ACT_EXP_FRAC = float(os.environ.get("MOE_ACT_EXP_FRAC", "0.62"))

_cache = {}


# ---------------------------------------------------------------------------
# Launch A
# ---------------------------------------------------------------------------
def _build_A():
    nc = bass.Bass("TRN2", target_bir_lowering=False, debug=False)

    xn8_i = nc.dram_tensor("xn8", [P, 4, 2, TOK], dt.float8e4, kind="ExternalInput").ap()
    wq8_i = nc.dram_tensor("wq8", [P, 4, 2, 3 * 512], dt.float8e4, kind="ExternalInput").ap()
    ow8_i = nc.dram_tensor("ow8", [P, 2, 2, E], dt.float8e4, kind="ExternalInput").ap()
    ao_o = nc.dram_tensor("ao", [P, 8, TOK], dt.float16, kind="ExternalOutput").ap()

    kq_dt = dt.float16 if QK16 else dt.float8e4

    with TileContext(nc) as tc:
        const = tc.alloc_tile_pool(name="const", bufs=1)
        ones64 = const.tile([1, 64], dt.float16)
        nc.vector.memset(ones64[:], 1.0)

        p_w = tc.alloc_tile_pool(name="p_w", bufs=1)
        wq8 = p_w.tile([P, 4, 2, 3 * 512], dt.float8e4)
        ow8 = p_w.tile([P, 2, 2, E], dt.float8e4)


        # engine-balance scheduler for PSUM-egress elementwise work
        ew_busy = {"act": 0.0, "dve": 0.0}

        def pick(rows, both=True):
            # projected per-engine cost: ACT 0.833/row +143ns, DVE 1.0417/row +125ns
            ca = rows * 0.833 + 143.0
            cd = rows * 1.0417 + 125.0
            if not both:
                ew_busy["dve"] += cd
                return "dve"
            if ew_busy["act"] + ca <= ew_busy["dve"] + cd:
                ew_busy["act"] += ca
                return "act"
            ew_busy["dve"] += cd
            return "dve"

        def egress_scale(out_ap, in_ap, scale, rows, force=None):
            """psum -> sbuf copy with scalar multiply, on the lighter engine."""
            eng = force or pick(rows)
            if eng == "act":
                nc.scalar.activation(out_ap, in_ap, AF.Copy, scale=scale)
            else:
                nc.vector.tensor_scalar(out_ap, in_ap, float(scale), None, op0=ALU.mult)
            return eng

        # ---- phase 1: QKV ----
        p_qkv = tc.alloc_tile_pool(name="p_qkv", bufs=1)
        _ = None
        if QK16:
            q8 = p_qkv.tile([P, 4, TOK], kq_dt)   # [p=2 heads x 64d, hp, t]
            k8 = p_qkv.tile([P, 4, TOK], kq_dt)
        else:
            q8 = p_qkv.tile([P, 2, 2, TOK], kq_dt)  # [p, dl, quad, t] (host-permuted W)
            k8 = p_qkv.tile([P, 2, 2, TOK], kq_dt)
        va = p_qkv.tile([P, KCP, 2, HL, 66], dt.float8e4)
        nc.gpsimd.memset(va[:, :, :, :, 64:66], 0.0)
        nc.gpsimd.memset(va[:, :, :, :, 64:65], 1.0)

        p_xn = tc.alloc_tile_pool(name="p_xn", bufs=1)
        xn8 = p_xn.tile([P, 4, 2, TOK], dt.float8e4)
        for c2 in range(4):
            for j in range(2):
                nc.sync.dma_start(xn8[:, c2, j, :], xn8_i[:, c2, j, :])
        for c2 in range(4):
            nc.sync.dma_start(wq8[:, c2, :, 1024:1536], wq8_i[:, c2, :, 1024:1536])
        for c2 in range(4):
            nc.sync.dma_start(wq8[:, c2, :, 0:1024], wq8_i[:, c2, :, 0:1024])
        nc.sync.dma_start(ow8[:], ow8_i)

        ps_q = tc.alloc_tile_pool(name="ps_q", bufs=4, space="PSUM")

        # v: token-major [128 tok, 512 vf] per token tile
        for tt in range(KC):
            pv = ps_q.tile([P, 512], dt.float32, tag="pq")
            for c2 in range(4):
                nc.tensor.matmul(pv[:], xn8[:, c2, :, tt * P:(tt + 1) * P],
                                 wq8[:, c2, :, 1024:1536],
                                 start=(c2 == 0), stop=(c2 == 3), perf_mode=PM.DoubleRow)
            egress_scale(
                va[:, tt // 2, tt % 2, :, 0:64],
                pv[:].rearrange("p (h d) -> p h d", d=64), 0.125, 512)

        # k then q: feature-major
        for which, qt in (("k", k8), ("q", q8)):
            base = 512 if which == "k" else 0
            for cc in range(4):
                for ts in range(4):
                    pk = ps_q.tile([P, 512], dt.float32, tag="pq")
                    for c2 in range(4):
                        nc.tensor.matmul(pk[:], wq8[:, c2, :, base + cc * P:base + (cc + 1) * P],
                                         xn8[:, c2, :, ts * 512:(ts + 1) * 512],
                                         start=(c2 == 0), stop=(c2 == 3), perf_mode=PM.DoubleRow)
                    if QK16:
                        dst = qt[:, cc, ts * 512:(ts + 1) * 512]
                    else:
                        dst = qt[:, cc % 2, cc // 2, ts * 512:(ts + 1) * 512]
                    egress_scale(dst, pk[:], 0.125, 512)
        p_xn.release()
        ps_q.release()

        # ---- phase 2: attention ----
        p_ctx = tc.alloc_tile_pool(name="p_ctx", bufs=1)
        ctx8 = p_ctx.tile([P, 2, 2, TOK], dt.float8e4)
        p_pr = tc.alloc_tile_pool(name="p_pr", bufs=int(os.environ.get("MOE_PR_BUFS", "3")))
        p_sb = tc.alloc_tile_pool(name="p_sb", bufs=3)
        ps_sc = tc.alloc_tile_pool(name="ps_sc", bufs=int(os.environ.get("MOE_SC_BUFS", "3")), space="PSUM")
        ps_ct = tc.alloc_tile_pool(name="ps_ct", bufs=int(os.environ.get("MOE_CT_BUFS", "1")), space="PSUM")

        n_exp = HL * 2 * KC
        exp_idx = [0]
        prts = {}

        def avdiv_ops(hl):
            """Closure list for head hl's AV + normalize, for interleaved emission."""
            prt = prts.pop(hl)
            ops = []
            state = {}

            def mk_av(qh, i, qs):
                def f():
                    if i == 0 and qs == 0:
                        state[qh] = ps_ct.tile([66, 1024], dt.float32, tag="ct", name="ct")
                    ct = state[qh]
                    sl = slice(qs * 512, qs * 512 + 512)
                    gl = slice(qh * 1024 + qs * 512, qh * 1024 + qs * 512 + 512)
                    nc.tensor.matmul(ct[:, sl], va[:, i, :, hl, :], prt[:, i, :, gl],
                                     start=(i == 0), stop=(i == KCP - 1),
                                     perf_mode=PM.DoubleRow)
                return f

            def mk_norm(qh):
                def f():
                    ct = state[qh]
                    rec = p_sb.tile([1, 1024], dt.float16, tag="rec", name="rec")
                    with nc.allow_low_precision(reason="softmax denominator reciprocal"):
                        nc.vector.reciprocal(rec[:], ct[64:65, :])
                    ew_busy["dve"] += 1024 * 1.0417 + 125.0
                    rb = ps_sc.tile([64, 1024], dt.float32, tag="sc", name="rb")
                    for qs in range(2):
                        sl = slice(qs * 512, qs * 512 + 512)
                        nc.tensor.matmul(rb[:, sl], ones64[:], rec[:, sl], start=True, stop=True)
                    rbs = p_sb.tile([64, 1024], dt.float16, tag="rbs", name="rbs")
                    eng = pick(1024)
                    if eng == "act":
                        nc.scalar.activation(rbs[:], rb[:], AF.Copy)
                    else:
                        nc.vector.tensor_copy(rbs[:], rb[:])
                    c2i, ji, pb = hl // 4, (hl // 2) % 2, 64 * (hl % 2)
                    nc.vector.scalar_tensor_tensor(
                        ctx8[pb:pb + 64, c2i, ji, qh * 1024:(qh + 1) * 1024],
                        ct[0:64, :], 16.0, rbs[:], op0=ALU.mult, op1=ALU.mult)
                    ew_busy["dve"] += 1024 * 1.0417 + 125.0
                return f

            for qh in range(2):
                for i in range(KCP):
                    for qs in range(2):
                        ops.append(mk_av(qh, i, qs))
                ops.append(mk_norm(qh))
            return ops

        pending = []

        def drain_pending(n):
            for _ in range(min(n, len(pending))):
                pending.pop(0)()

        for hl in range(HL):
            prt = p_pr.tile([P, KCP, 2, TOK], dt.float8e4, tag="pr", name="prt")
            prts[hl] = prt
            for qh in range(2):
                for kc in range(KC):
                    sc = ps_sc.tile([P, 1024], dt.float32, tag="sc")
                    for qs in range(2):
                        sl = slice(qs * 512, qs * 512 + 512)
                        gl = slice(qh * 1024 + qs * 512, qh * 1024 + qs * 512 + 512)
                        if QK16:
                            b0 = 64 * (hl % 2)
                            nc.tensor.matmul(sc[:, sl], k8[b0:b0 + 64, hl // 2, kc * P:(kc + 1) * P],
                                             q8[b0:b0 + 64, hl // 2, gl],
                                             start=True, stop=True,
                                             tile_position=(b0, 0))
                        else:
                            b0 = 32 * (hl % 4)
                            nc.tensor.matmul(sc[:, sl], k8[b0:b0 + 32, :, hl // 4, kc * P:(kc + 1) * P],
                                             q8[b0:b0 + 32, :, hl // 4, gl],
                                             start=True, stop=True, perf_mode=PM.DoubleRow,
                                             tile_position=(b0, 0))
                    # exp: ACT exact or DVE Schraudolph, balanced split
                    dst = prt[:, kc // 2, kc % 2, qh * 1024:(qh + 1) * 1024]
                    # greedy balance: ACT exact exp vs DVE Schraudolph
                    ca = 1024 * 0.833 + 143.0
                    cd = 1024 * 1.0417 + 125.0
                    if ew_busy["act"] + ca <= ew_busy["dve"] + cd:
                        nc.scalar.activation(dst, sc[:], AF.Exp, scale=0.125)
                        ew_busy["act"] += ca
                    else:
                        # Schraudolph: byte = s*log2e + 56 -> bitcast fp8e4 = exp(s/8)
                        nc.vector.tensor_scalar(dst.bitcast(dt.uint8), sc[:],
                                                LOG2E, 56.0, op0=ALU.mult, op1=ALU.add)
                        ew_busy["dve"] += cd
                    drain_pending(1 if kc % 8 else 2)

            if hl > 0:
                pending.extend(avdiv_ops(hl - 1))
        while pending:
            pending.pop(0)()
        for op in avdiv_ops(HL - 1):
            op()

        ps_ct.release()
        ps_sc.release()
        p_sb.release()
        p_pr.release()

        # ---- phase 3: partial out-proj ----
        ps_ao = tc.alloc_tile_pool(name="ps_ao", bufs=3, space="PSUM")
        p_ac = tc.alloc_tile_pool(name="p_ac", bufs=2)
        for eo in range(8):
            aos = p_ac.tile([P, TOK], dt.float16, tag="aos")
            for qs in range(4):
                sl = slice(qs * 512, qs * 512 + 512)
                pao = ps_ao.tile([P, 512], dt.float32, tag="ao")
                for c2 in range(2):
                    nc.tensor.matmul(pao[:], ow8[:, c2, :, eo * P:(eo + 1) * P],
                                     ctx8[:, c2, :, sl],
                                     start=(c2 == 0), stop=(c2 == 1), perf_mode=PM.DoubleRow)
                egress_scale(aos[:, sl], pao[:], 1.0, 512)
            nc.sync.dma_start(ao_o[:, eo, :], aos[:])
        p_ac.release()
        ps_ao.release()
        p_ctx.release()
        p_qkv.release()
        p_w.release()
        const.release()

    return nc


# ---------------------------------------------------------------------------
# Launch B
# ---------------------------------------------------------------------------
def _build_B():
    nc = bass.Bass("TRN2", target_bir_lowering=False, debug=False)
    xe_i = nc.dram_tensor("xe", [P, 4, 2, C], dt.float8e4, kind="ExternalInput").ap()
    w1_i = nc.dram_tensor("w1e", [P, 4, 2, FF], dt.float8e4, kind="ExternalInput").ap()
    w2_i = nc.dram_tensor("w2e", [P, 16, 2, E], dt.float8e4, kind="ExternalInput").ap()
    b1_i = nc.dram_tensor("b1e", [P, FF // P], dt.float32, kind="ExternalInput").ap()
    if XESPLIT:
        xl_i = nc.dram_tensor("xel", [P, 4, 2, C], dt.float8e4, kind="ExternalInput").ap()
    if W2SPLIT:
        w2l_i = nc.dram_tensor("w2l", [P, 16, 2, E], dt.float8e4, kind="ExternalInput").ap()
    o_o = nc.dram_tensor("o", [P, CT, E], dt.float16, kind="ExternalOutput").ap()

    with TileContext(nc) as tc:
        sbw = tc.alloc_tile_pool(name="sbw", bufs=1)
        xe = sbw.tile([P, 4, 2, C], dt.float8e4)
        w1 = sbw.tile([P, 4, 2, FF], dt.float8e4)
        w2 = sbw.tile([P, 16, 2, E], dt.float8e4)
        b1 = sbw.tile([P, FF // P], dt.float32)
        nc.sync.dma_start(b1[:], b1_i)
        xl = w2l = None
        if XESPLIT:
            xl = sbw.tile([P, 4, 2, C], dt.float8e4)
        if W2SPLIT:
            w2l = sbw.tile([P, 16, 2, E], dt.float8e4)
        # first block operands first
        for c2 in range(4):
            nc.sync.dma_start(xe[:, c2, :, 0:512], xe_i[:, c2, :, 0:512])
            if XESPLIT:
                nc.sync.dma_start(xl[:, c2, :, 0:512], xl_i[:, c2, :, 0:512])
        for c2 in range(4):
            for j in range(2):
                nc.sync.dma_start(w1[:, c2, j, 0:2048], w1_i[:, c2, j, 0:2048])
        for c2 in range(4):
            nc.sync.dma_start(xe[:, c2, :, 512:C], xe_i[:, c2, :, 512:C])
            if XESPLIT:
                nc.sync.dma_start(xl[:, c2, :, 512:C], xl_i[:, c2, :, 512:C])
        for c2 in range(4):
            for j in range(2):
                nc.sync.dma_start(w1[:, c2, j, 2048:FF], w1_i[:, c2, j, 2048:FF])
        for fcp in range(0, 16, 4):
            nc.sync.dma_start(w2[:, fcp:fcp + 4, :, :], w2_i[:, fcp:fcp + 4, :, :])
            if W2SPLIT:
                nc.sync.dma_start(w2l[:, fcp:fcp + 4, :, :], w2l_i[:, fcp:fcp + 4, :, :])

        p_hs = tc.alloc_tile_pool(name="p_hs", bufs=2)
        p_os = tc.alloc_tile_pool(name="p_os", bufs=3)
        ps_h = tc.alloc_tile_pool(name="ps_h", bufs=4, space="PSUM")
        ps_o = tc.alloc_tile_pool(name="ps_o", bufs=2, space="PSUM")

        blocks = []
        t0 = 0
        while t0 < CT:
            nt = min(4, CT - t0)
            blocks.append((t0, nt))
            t0 += nt

        for (t0, nt) in blocks:
            ntok = nt * P
            hs2 = p_hs.tile([P, 16, 2, 512], dt.float8e4, tag="hs2")
            for fc in range(FF // P):
                ph = ps_h.tile([P, 512], dt.float32, tag="ph")
                nsl = ((ntok + 511) // 512)
                for ss in range(nsl):
                    w = min(512, ntok - ss * 512)
                    sl = slice(ss * 512, ss * 512 + w)
                    gsl = slice(t0 * P + ss * 512, t0 * P + ss * 512 + w)
                    for c2 in range(4):
                        nc.tensor.matmul(ph[:, sl], w1[:, c2, :, fc * P:(fc + 1) * P],
                                         xe[:, c2, :, gsl],
                                         start=(c2 == 0),
                                         stop=(c2 == 3) and not XESPLIT,
                                         perf_mode=PM.DoubleRow)
                    if XESPLIT:
                        for c2 in range(4):
                            nc.tensor.matmul(ph[:, sl], w1[:, c2, :, fc * P:(fc + 1) * P],
                                             xl[:, c2, :, gsl],
                                             start=False, stop=(c2 == 3),
                                             perf_mode=PM.DoubleRow)
                nc.scalar.activation(hs2[:, fc // 2, fc % 2, 0:ntok], ph[:, 0:ntok],
                                     AF.Gelu, bias=b1[:, fc:fc + 1], scale=0.125)
            for tt in range(nt):
                po = ps_o.tile([P, 1024], dt.float32, tag="po")
                for fcp in range(16):
                    lhs = hs2[:, fcp, :, tt * P:(tt + 1) * P]
                    for es in range(2):
                        sl = slice(es * 512, es * 512 + 512)
                        nc.tensor.matmul(po[:, sl], lhs, w2[:, fcp, :, sl],
                                         start=(fcp == 0),
                                         stop=(fcp == 15) and not W2SPLIT,
                                         perf_mode=PM.DoubleRow)
                if W2SPLIT:
                    for fcp in range(16):
                        lhs = hs2[:, fcp, :, tt * P:(tt + 1) * P]
                        for es in range(2):
                            sl = slice(es * 512, es * 512 + 512)
                            nc.tensor.matmul(po[:, sl], lhs, w2l[:, fcp, :, sl],
                                             start=False, stop=(fcp == 15),
                                             perf_mode=PM.DoubleRow)
                os_ = p_os.tile([P, 1024], dt.float16, tag="os")
                nc.vector.tensor_copy(os_[:], po[:])
                nc.sync.dma_start(o_o[:, t0 + tt, :], os_[:])

        p_os.release()
        p_hs.release()
        ps_o.release()
        ps_h.release()
        sbw.release()

    return nc


# ---------------------------------------------------------------------------
# Host-side helpers
# ---------------------------------------------------------------------------
def _f8(a):
    return np.ascontiguousarray(a).astype(F8)


def _dr_feat(a):
    """[E, N] feature-major -> DR layout [P, E//256, 2, N]."""
    Edim, N = a.shape
    return np.ascontiguousarray(a.reshape(Edim // 256, 2, P, N).transpose(2, 0, 1, 3))


def _qk_perm(g):
    """Feature permutation for q/k blocks (fp8 scores layout).

    chunk cc = quad*2 + dl, col p -> head (quad*4 + p//32), dim (dl*32 + p%32).
    Returns index array idx[512] into the core's local q/k feature range.
    """
    idx = np.empty(512, dtype=np.int64)
    f = 0
    for quad in range(2):
        for dl in range(2):
            for p in range(128):
                h = quad * 4 + p // 32
                d = dl * 32 + (p % 32)
                idx[f] = h * 64 + d
                f += 1
    # note: quad covers heads 0..3 (quad 0) and 4..7 (quad 1)
    return idx


def kernel(**inputs):
    x = np.asarray(inputs["x"], dtype=np.float32)
    in_proj_w = np.asarray(inputs["in_proj_w"], dtype=np.float32)
    in_proj_b = np.asarray(inputs["in_proj_b"], dtype=np.float32)
    out_w = np.asarray(inputs["out_w"], dtype=np.float32)
    out_b = np.asarray(inputs["out_b"], dtype=np.float32)
    ln1_g = np.asarray(inputs["ln1_g"], dtype=np.float32)
    ln1_b = np.asarray(inputs["ln1_b"], dtype=np.float32)
    ln2_g = np.asarray(inputs["ln2_g"], dtype=np.float32)
    ln2_b = np.asarray(inputs["ln2_b"], dtype=np.float32)
    gate_w = np.asarray(inputs["gate_w"], dtype=np.float32)
    gate_b = np.asarray(inputs["gate_b"], dtype=np.float32)
    w1 = np.asarray(inputs["w1"], dtype=np.float32)
    b1 = np.asarray(inputs["b1"], dtype=np.float32)
    w2 = np.asarray(inputs["w2"], dtype=np.float32)
    b2 = np.asarray(inputs["b2"], dtype=np.float32)

    assert np.all(in_proj_b == 0.0), "nonzero in_proj_b unsupported"

    if "A" not in _cache:
        _cache["A"] = _build_A()
    if "B" not in _cache:
        _cache["B"] = _build_B()
    ncA, ncB = _cache["A"], _cache["B"]

    # ---- host: LN1 ----
    x64 = x.astype(np.float64)
    mu1 = x64.mean(-1, keepdims=True)
    rs1 = 1.0 / np.sqrt(x64.var(-1) + LN_EPS)
    xn = ((x64 - mu1) * rs1[..., None]).astype(np.float32) * ln1_g + ln1_b  # [S,B,E]

    # ---- launch A host prep ----
    qk_idx = _qk_perm(0)
    in_maps_A = []
    wq8_g, ow8_g = {}, {}
    for g in range(2):
        rows = []
        base = g * 512
        if QK16:
            rows.append(in_proj_w[base:base + 512, :])                 # q natural order
            rows.append(in_proj_w[E + base:E + base + 512, :])         # k natural
        else:
            rows.append(in_proj_w[base:base + 512, :][qk_idx])         # q permuted
            rows.append(in_proj_w[E + base:E + base + 512, :][qk_idx])  # k permuted
        rows.append(in_proj_w[2 * E + base:2 * E + base + 512, :])     # v natural
        Wg = np.concatenate(rows, axis=0)          # [1536, E]
        wq8_g[g] = _f8(
            np.ascontiguousarray(Wg.T * 8.0).reshape(4, 2, P, 1536).transpose(2, 0, 1, 3))
        # ow: ctx slot (c2, j, p) -> head hl = c2*4 + j*2 + p//64, d = p%64
        ow_cols = np.empty((512,), dtype=np.int64)
        for c2 in range(2):
            for j in range(2):
                for p in range(128):
                    hl = c2 * 4 + j * 2 + p // 64
                    d = p % 64
                    ow_cols[c2 * 256 + j * 128 + p] = base + hl * 64 + d
        owT = out_w[:, ow_cols].T * 128.0          # [512 ctx-slots, E]
        ow8_g[g] = _f8(owT.reshape(2, 2, P, E).transpose(2, 0, 1, 3))

    for c in range(NCORES):
        b, g = c // 2, c % 2
        xnT = np.ascontiguousarray(xn[:, b, :].T)  # [E, TOK]
        xn8 = _f8(xnT.reshape(4, 2, P, TOK).transpose(2, 0, 1, 3))
        in_maps_A.append({"xn8": xn8, "wq8": wq8_g[g], "ow8": ow8_g[g]})

    resA = run_bass_kernel_spmd(ncA, in_maps_A, core_ids=list(range(NCORES)))
    outsA = resA.results

    # ---- host: residual, LN2, routing ----
    T = S * B
    x1 = x64.copy()  # [S,B,E] float64
    for c in range(NCORES):
        b = c // 2
        ao = outsA[c]["ao"].astype(np.float32)  # [P, 8, TOK] fp16, scale 2048
        aoF = ao.transpose(1, 0, 2).reshape(E, TOK)
        x1[:, b, :] += aoF.T.astype(np.float64) * (1.0 / 2048.0)

    x1 += out_b.astype(np.float64)[None, None, :]
    mu2 = x1.mean(-1, keepdims=True)
    var2 = x1.var(-1)
    rs2 = 1.0 / np.sqrt(var2 + LN_EPS)
    xn2 = ((x1 - mu2) * rs2[..., None]) * ln2_g.astype(np.float64) + ln2_b.astype(np.float64)
    xn2f = xn2.reshape(T, E)                          # [T,E] float64, token order t = s*B + b
    logits = xn2f @ gate_w.astype(np.float64).T + gate_b.astype(np.float64)

    i1 = np.argmax(logits, axis=1)
    l2m = logits.copy()
    l2m[np.arange(T), i1] = -np.inf
    i2 = np.argmax(l2m, axis=1)
    v1 = logits[np.arange(T), i1]
    v2 = logits[np.arange(T), i2]
    e2 = np.exp(v2 - v1)
    gsc1 = (1.0 / (1.0 + e2)).astype(np.float32)
    gsc2 = (e2 / (1.0 + e2)).astype(np.float32)

    expert_rows, expert_w = [], []
    for e in range(NE):
        m1 = i1 == e
        m2 = i2 == e
        rows = np.nonzero(m1 | m2)[0]
        w = np.where(m1[rows], gsc1[rows], gsc2[rows]).astype(np.float32)
        if len(rows) > C:   # capacity safeguard: drop lowest-weight assignments
            keep = np.sort(np.argsort(-w)[:C])
            rows, w = rows[keep], w[keep]
        expert_rows.append(rows)
        expert_w.append(w)

    # ---- launch B host prep ----
    xn2f32 = xn2f.astype(np.float32)
    in_maps_B = []
    for e in range(NE):
        rows = expert_rows[e]
        xeT = np.zeros((E, C), dtype=np.float32)
        xeT[:, :len(rows)] = xn2f32[rows].T
        xe_hi = xeT.astype(F8)
        m = {
            "xe": np.ascontiguousarray(
                xe_hi.reshape(4, 2, P, C).transpose(2, 0, 1, 3)),
            "w1e": _f8((w1[e] * 8.0).reshape(4, 2, P, FF).transpose(2, 0, 1, 3)),
            "w2e": _f8((w2[e] * 8.0).reshape(16, 2, P, E).transpose(2, 0, 1, 3)),
            "b1e": np.ascontiguousarray(b1[e].reshape(FF // P, P).T),
        }
        if XESPLIT:
            xlo = xeT - xe_hi.astype(np.float32)   # unscaled residual; fp8 subnormals suffice
            m["xel"] = _f8(xlo.reshape(4, 2, P, C).transpose(2, 0, 1, 3))
        if W2SPLIT:
            w2s = w2[e] * 8.0
            w2hi = w2s.astype(F8).astype(np.float32)
            w2lo = w2s - w2hi
            m["w2l"] = _f8(w2lo.reshape(16, 2, P, E).transpose(2, 0, 1, 3))
        in_maps_B.append(m)

    resB = run_bass_kernel_spmd(ncB, in_maps_B, core_ids=list(range(NCORES)))
    outsB = resB.results

    # ---- host combine ----
    y = np.zeros((T, E), dtype=np.float32)
    for e in range(NE):
        rows, w = expert_rows[e], expert_w[e]
        o = outsB[e]["o"].astype(np.float32).transpose(1, 0, 2).reshape(C, E)
        oc = o * (1.0 / 8.0)
        y[rows] += w[:, None] * oc[:len(rows)]
        if np.any(b2[e] != 0.0):
            y[rows] += w[:, None] * b2[e][None, :]

    out = x1.reshape(T, E).astype(np.float32) + y
    return out.reshape(S, B, E)
